# revision 8
# baseline (speedup 1.0000x reference)
"""BiLSTM diacritizer Trainium2 kernel.

8 NeuronCores, SPMD, identical program, zero collectives.
Core c -> batch row b=c//2, attention t-half th=c%2.
Each core computes its batch row's full 3-layer BiLSTM (fwd+bwd chains
interleaved), then Bahdanau attention + classifier for its 128 query
positions. Host pre-permutes/casts weights and assembles the output.

Recurrence design: gate pre-activations gx for a whole layer are
accumulated directly in PSUM (all 8 banks: [128, 2dir, 8mc, 256t] f32),
bias injected via a tiny selector matmul, and the per-step h@Whh GEMVs
run as fp8e4 DoubleRow matmuls (K=256 contracted per instruction, so 8
matmuls per dir-step instead of 17). h is carried in fp8 for the
recurrence and bulk-cast to f16 once per layer for the next layer's
input GEMM and the attention block. Weights/bias are pre-scaled x16 on
the host so fp8 stays in the normal range; the gate sigmoid applies
scale=1/16.
"""

import sys

sys.path.insert(0, "/opt/trn_rl_repo")

from contextlib import ExitStack

import numpy as np
import ml_dtypes

import concourse.bacc as bacc
import concourse.bass as bass
import concourse.tile as tile
from concourse import mybir

# Model dims (hardcoded per problem spec)
V, E, H, C = 64, 128, 256, 15
H2 = 2 * H          # 512
G = 4 * H           # 1024 gate width
B, S = 4, 256
N_CORES = 8
NL = 3              # LSTM layers
MC = G // 128       # 8 gate-dim chunks
KC_H = H // 128     # 2 h-dim chunks
KC_H2 = H2 // 128   # 4 chunks of the 512-dim layer input / hidden concat
GSC = 16.0          # gate pre-activation scale (wih/whh/bias x16 on host)

F32 = mybir.dt.float32
F16 = mybir.dt.float16
F8 = mybir.dt.float8e4
AF = mybir.ActivationFunctionType
OP = mybir.AluOpType
DR = mybir.MatmulPerfMode.DoubleRow

# Gate permutation: torch order i,f,g,o -> device order i,f,o,g
# (so sigmoid covers contiguous chunks 0..5, tanh chunks 6..7)
_PERM = np.concatenate([
    np.arange(0, 256), np.arange(256, 512), np.arange(768, 1024),
    np.arange(512, 768),
])


def _build_nc(nl=NL, s_steps=S):
    """Build the SPMD program. nl/s_steps shrinkable for fast testing."""
    nc = bacc.Bacc(None, target_bir_lowering=False, num_devices=N_CORES)

    # ---- external inputs (per-core data, same names everywhere) ----
    d = {}
    d["ids"] = nc.dram_tensor("ids", [1, S], F32, kind="ExternalInput")
    d["sel"] = nc.dram_tensor("sel", [128, 2, 128], F16, kind="ExternalInput")
    d["embT"] = nc.dram_tensor("embT", [V, E], F16, kind="ExternalInput")
    d["wih0T"] = nc.dram_tensor("wih0T", [128, 2, MC, 128], F16,
                                kind="ExternalInput")
    d["wihT"] = nc.dram_tensor("wihT", [128, 2, 2, KC_H2, MC, 128], F16,
                               kind="ExternalInput")
    d["whh8"] = nc.dram_tensor("whh8", [128, NL, 2, MC, KC_H, 128], F8,
                               kind="ExternalInput")
    d["biasT"] = nc.dram_tensor("biasT", [2, NL, 2, MC // 2, 128], F16,
                                kind="ExternalInput")
    d["attnT"] = nc.dram_tensor("attnT", [128, 2, KC_H2, KC_H2, 128], F16,
                                kind="ExternalInput")
    d["vT"] = nc.dram_tensor("vT", [128, KC_H2], F16, kind="ExternalInput")
    d["vsel"] = nc.dram_tensor("vsel", [128, KC_H2, 32, 32], F16,
                               kind="ExternalInput")
    d["clsWT"] = nc.dram_tensor("clsWT", [128, KC_H2, C], F16,
                                kind="ExternalInput")
    d["clsb"] = nc.dram_tensor("clsb", [C, 1], F32, kind="ExternalInput")
    d["id16"] = nc.dram_tensor("id16", [128, 128], F16, kind="ExternalInput")
    d["id32"] = nc.dram_tensor("id32", [128, 128], F32, kind="ExternalInput")
    d["ones2"] = nc.dram_tensor("ones2", [2, 2, 256], F16,
                                kind="ExternalInput")
    d["out"] = nc.dram_tensor("logitsT", [C, 128], F32, kind="ExternalOutput")

    with tile.TileContext(nc) as tc, ExitStack() as ctx:
        _emit(ctx, tc, nc, nl, s_steps, d)
    nc.compile()
    return nc


def _emit(ctx, tc, nc, nl, SS, d):
    fp = ctx.enter_context(tc.tile_pool(name="persist", bufs=1))

    # ---- load constants to SBUF, ordered so layer 0 can start ASAP ----
    def _alloc(name, shape, dtype):
        return fp.tile(shape, dtype, name=f"sb_{name}", tag=f"sb_{name}")

    def _dma(t, name, sl=None):
        if sl is None:
            nc.gpsimd.dma_start(out=t[:], in_=d[name][:])
        else:
            nc.gpsimd.dma_start(out=t[:, sl], in_=d[name][:, sl])

    emb_sb = _alloc("embT", [V, E], F16)
    wih0_sb = _alloc("wih0T", [128, 2, MC, 128], F16)
    whh8_sb = _alloc("whh8", [128, NL, 2, MC, KC_H, 128], F8)
    bias_sb = _alloc("biasT", [2, NL, 2, MC // 2, 128], F16)
    ones2_sb = _alloc("ones2", [2, 2, 256], F16)
    wih_sb = _alloc("wihT", [128, 2, 2, KC_H2, MC, 128], F16)
    attn_sb = _alloc("attnT", [128, 2, KC_H2, KC_H2, 128], F16)
    v_sb = _alloc("vT", [128, KC_H2], F16)
    vsel_sb = _alloc("vsel", [128, KC_H2, 32, 32], F16)
    clsw_sb = _alloc("clsWT", [128, KC_H2, C], F16)
    clsb_sb = _alloc("clsb", [C, 1], F32)
    id16_sb = _alloc("id16", [128, 128], F16)
    id32_sb = _alloc("id32", [128, 128], F32)
    sel_sb = _alloc("sel", [128, 2, 128], F16)

    # early: everything layer 0 needs
    _dma(emb_sb, "embT")
    _dma(wih0_sb, "wih0T")
    _dma(bias_sb, "biasT")
    _dma(ones2_sb, "ones2")
    _dma(whh8_sb, "whh8", 0)
    _dma(whh8_sb, "whh8", 1)
    _dma(wih_sb, "wihT", 0)
    _dma(whh8_sb, "whh8", 2)
    _dma(wih_sb, "wihT", 1)
    # late: attention/classifier-phase tensors
    _dma(id16_sb, "id16")
    _dma(attn_sb, "attnT")
    _dma(v_sb, "vT")
    _dma(vsel_sb, "vsel")
    _dma(clsw_sb, "clsWT")
    _dma(clsb_sb, "clsb")
    _dma(id32_sb, "id32")
    _dma(sel_sb, "sel")

    # fp8 DoubleRow rhs ISA rules: (DR-dim, N-dim) free APs, DR-dim step
    # 16B-aligned, start offset 2B-aligned. zeros8 sliced [:, :, 0:1].
    zeros8 = fp.tile([128, KC_H, 16], F8)
    nc.vector.memset(zeros8[:], 0.0)

    # ---- embedding: one-hot matmul -> xT [E=128, S] f16 ----
    ids_ap = d["ids"].ap()
    ids_b = fp.tile([V, S], F32)
    nc.gpsimd.dma_start(
        out=ids_b[:],
        in_=bass.AP(tensor=ids_ap.tensor, offset=ids_ap.offset,
                    ap=[[0, V], [1, S]]),
    )
    iota_i = fp.tile([V, 1], mybir.dt.int32)
    nc.gpsimd.iota(iota_i[:], pattern=[[0, 1]], base=0, channel_multiplier=1)
    iota_f = fp.tile([V, 1], F32)
    nc.vector.tensor_copy(iota_f[:], iota_i[:])
    oh = fp.tile([V, S], F16)
    nc.vector.tensor_scalar(
        out=oh[:], in0=ids_b[:], scalar1=iota_f[:], scalar2=None,
        op0=OP.is_equal,
    )
    xT_sb = fp.tile([128, 1, S], F16)   # layer-0 input, 1 k-chunk
    with tc.tile_pool(name="embp", bufs=1, space="PSUM") as embp:
        x_ps = embp.tile([128, S], F32)
        nc.tensor.matmul(x_ps[:], emb_sb[:], oh[:], start=True, stop=True)
        nc.vector.tensor_copy(xT_sb[:, 0, :], x_ps[:])

    # ---- LSTM layers ----
    h8_pool = ctx.enter_context(tc.tile_pool(name="h8", bufs=2))
    h16_pool = ctx.enter_context(tc.tile_pool(name="h16", bufs=2))
    prev = xT_sb          # [128, kc_in, S] f16
    kc_in = 1
    for layer in range(nl):
        wl = wih0_sb if layer == 0 else wih_sb
        # h for the recurrence, fp8, t-stride padded to 2B so every
        # column offset is even; [128, dir, kc, t, pad2]
        hT8 = h8_pool.tile([128, 2, KC_H, S, 2], F8, tag="hT8")
        with (
            tc.tile_pool(name=f"gps{layer}", bufs=1, space="PSUM") as gps,
            tc.tile_pool(name=f"rsb{layer}", bufs=4) as rsb,
        ):
            # gate pre-activations, whole layer, in PSUM (all 8 banks)
            g_ps = gps.tile([128, 2, MC, S], F32)
            # bias via selector matmul: one per (dir, mc-pair) -> 1 bank
            for dd in (0, 1):
                for p in range(MC // 2):
                    nc.tensor.matmul(
                        g_ps[:, dd, 2 * p:2 * p + 2, :],
                        bias_sb[:, layer, dd, p, :], ones2_sb[:],
                        start=True, stop=True, skip_group_check=True,
                    )
            # input GEMM accumulates on top
            for dd in (0, 1):
                for mc in range(MC):
                    for kc in range(kc_in):
                        if layer == 0:
                            lhsT = wl[:, dd, mc, :]
                        else:
                            lhsT = wl[:, layer - 1, dd, kc, mc, :]
                        nc.tensor.matmul(
                            g_ps[:, dd, mc, :], lhsT, prev[:, kc, :],
                            start=False, stop=(kc == kc_in - 1),
                            skip_group_check=True,
                        )
            # recurrence: fwd (d=0) + bwd (d=1) interleaved chains.
            # 8 fp8 DoubleRow matmuls per dir-step contract K=256 each.
            combo = [None, None]
            for dd in (0, 1):
                c0 = rsb.tile([128, 4], F32, tag=f"combo{dd}")
                nc.vector.memset(c0[:], 0.0)
                combo[dd] = c0
            for t in range(SS):
                for dd in (0, 1):
                    tg = t if dd == 0 else SS - 1 - t
                    tprev = tg + 1 if dd == 1 else tg - 1
                    if t == 0:
                        rhs = zeros8[:, :, 0:1]
                    else:
                        rhs = hT8[:, dd, :, tprev, 0:1]
                    for mc in range(MC):
                        nc.tensor.matmul(
                            g_ps[:, dd, mc, tg:tg + 1],
                            whh8_sb[:, layer, dd, mc, :, :], rhs,
                            start=False, stop=(mc == MC - 1),
                            perf_mode=DR, skip_group_check=True,
                        )
                    # gates: i=0:2 f=2:4 o=4:6 g=6:8 (chunk cols).
                    # g-rows pre-scaled x2 host-side: tanh(g)=2*sig(2g)-1,
                    # so one sigmoid covers all gates. All gate weights
                    # carry x16; undo via the activation scale.
                    s_ifo = rsb.tile([128, 8], F32, tag=f"s{dd}")
                    nc.scalar.activation(s_ifo[:], g_ps[:, dd, :, tg],
                                         AF.Sigmoid, scale=1.0 / GSC)
                    cmb = combo[dd]
                    nc.vector.tensor_scalar(
                        out=cmb[:, 0:2], in0=s_ifo[:, 6:8], scalar1=2.0,
                        scalar2=-1.0, op0=OP.mult, op1=OP.add)
                    prods = rsb.tile([128, 4], F32, tag=f"p{dd}")
                    nc.vector.tensor_mul(prods[:], s_ifo[:, 0:4], cmb[:])
                    cmb_n = rsb.tile([128, 4], F32, tag=f"combo{dd}")
                    nc.vector.tensor_add(cmb_n[:, 2:4], prods[:, 0:2],
                                         prods[:, 2:4])
                    combo[dd] = cmb_n
                    # tanh(c)=2*sig(2c)-1 via free ACT scale; emit h/2 =
                    # (sig-0.5)*o; all h consumers are doubled host-side.
                    tc_t = rsb.tile([128, 2], F32, tag=f"tc{dd}")
                    nc.scalar.activation(tc_t[:], cmb_n[:, 2:4], AF.Sigmoid,
                                         scale=2.0)
                    nc.vector.scalar_tensor_tensor(
                        out=hT8[:, dd, :, tg, 0], in0=tc_t[:],
                        scalar=0.5, in1=s_ifo[:, 4:6],
                        op0=OP.subtract, op1=OP.mult)
        # bulk-cast h to f16 for the next layer's input GEMM / attention
        hT16 = h16_pool.tile([128, 4, S], F16, tag="hT16")
        nc.vector.tensor_copy(hT16[:], hT8[:, :, :, :, 0])
        prev = hT16
        kc_in = KC_H2

    # ---- attention + classifier ----
    hT = prev  # [128, 4, S] f16 final hidden (transposed layout)
    ap1 = ctx.enter_context(tc.tile_pool(name="attn1", bufs=1))

    # h layout [s, h]: PE-transpose hT blocks -> h_sb[:, sc, hc, :]
    h_sb = ap1.tile([128, 2, KC_H2, 128], F16)
    with tc.tile_pool(name="trps", bufs=4, space="PSUM") as trps:
        for hc in range(KC_H2):
            for sc in range(2):
                tp = trps.tile([128, 128], F16, tag="tp")
                nc.tensor.transpose(tp[:], hT[:, hc, sc * 128:(sc + 1) * 128],
                                    id16_sb[:])
                nc.vector.tensor_copy(h_sb[:, sc, hc, :], tp[:])

    # hT_sel [h, tl] = h-cols for my t-half, via Sel matmul; then q, k
    hsel_sb = ap1.tile([128, KC_H2, 128], F16)
    qT_sb = ap1.tile([128, KC_H2, 128], F32)
    with ExitStack() as actx:
        kTp = actx.enter_context(tc.tile_pool(name="kTps", bufs=1, space="PSUM"))
        kT_ps = kTp.tile([128, KC_H2, S], F32)
        with tc.tile_pool(name="qkps", bufs=2, space="PSUM") as qkps:
            for hc in range(KC_H2):
                ps = qkps.tile([128, 128], F32, tag="sel")
                for sc in range(2):
                    nc.tensor.matmul(ps[:], h_sb[:, sc, hc, :],
                                     sel_sb[:, sc, :],
                                     start=(sc == 0), stop=(sc == 1))
                nc.vector.tensor_copy(hsel_sb[:, hc, :], ps[:])
            for mc in range(KC_H2):
                psq = qkps.tile([128, 128], F32, tag="q")
                for kc in range(KC_H2):
                    nc.tensor.matmul(psq[:], attn_sb[:, 0, kc, mc, :],
                                     hsel_sb[:, kc, :],
                                     start=(kc == 0), stop=(kc == KC_H2 - 1))
                nc.vector.tensor_copy(qT_sb[:, mc, :], psq[:])
            for mc in range(KC_H2):
                for kc in range(KC_H2):
                    nc.tensor.matmul(kT_ps[:, mc, :], attn_sb[:, 1, kc, mc, :],
                                     hT[:, kc, :],
                                     start=(kc == 0), stop=(kc == KC_H2 - 1))

        # scores[t, s] = sum_hc vT . tanh(kT + q[t]); 32 t-rows per psum
        # tile via v-selector lhsT (v in col t%32, zero rows accumulate 0)
        scp = actx.enter_context(tc.tile_pool(name="scps", bufs=2, space="PSUM"))
        scores_sb = ap1.tile([128, S], F32)
        with tc.tile_pool(name="tanhp", bufs=4) as tanhp:
            for tg_i in range(4):
                sc_ps = scp.tile([32, S], F32, tag="sc")
                for tj in range(32):
                    t = tg_i * 32 + tj
                    for hc in range(KC_H2):
                        th_t = tanhp.tile([128, S], F16, tag="th")
                        nc.scalar.activation(th_t[:], kT_ps[:, hc, :], AF.Tanh,
                                             bias=qT_sb[:, hc, t:t + 1])
                        nc.tensor.matmul(
                            sc_ps[:], vsel_sb[:, hc, tj, :], th_t[:],
                            start=(tj == 0 and hc == 0),
                            stop=(tj == 31 and hc == KC_H2 - 1),
                            skip_group_check=True,
                        )
                nc.vector.tensor_copy(
                    scores_sb[tg_i * 32:(tg_i + 1) * 32, :], sc_ps[:])

        # softmax rows -> wn_sb [128, S] f32
        ap2 = ctx.enter_context(tc.tile_pool(name="attn2", bufs=1))
        wn_sb = ap2.tile([128, S], F32)
        nmax = ap2.tile([128, 1], F32)
        nc.vector.tensor_reduce(out=nmax[:], in_=scores_sb[:], op=OP.max,
                                axis=mybir.AxisListType.X, negate=True)
        rsum = ap2.tile([128, 1], F32)
        wexp = ap2.tile([128, S], F32)
        nc.scalar.activation(wexp[:], scores_sb[:], AF.Exp,
                             bias=nmax[:], accum_out=rsum[:])
        rinv = ap2.tile([128, 1], F32)
        nc.vector.reciprocal(rinv[:], rsum[:])
        nc.vector.tensor_scalar_mul(wn_sb[:], wexp[:], rinv[:])

    # wT via PE transpose -> f16; ctxT; classifier
    ap2b = ctx.enter_context(tc.tile_pool(name="attn3", bufs=1))
    wT_sb = ap2b.tile([128, 2, 128], F16)
    ctxT_sb = ap2b.tile([128, KC_H2, 128], F16)
    with tc.tile_pool(name="ctps", bufs=2, space="PSUM") as ctps:
        for sc in range(2):
            tp32 = ctps.tile([128, 128], F32, tag="wt")
            nc.tensor.transpose(tp32[:], wn_sb[:, sc * 128:(sc + 1) * 128],
                                id32_sb[:])
            nc.vector.tensor_copy(wT_sb[:, sc, :], tp32[:])
        for hc in range(KC_H2):
            ps = ctps.tile([128, 128], F32, tag="ctx")
            for sc in range(2):
                nc.tensor.matmul(ps[:], h_sb[:, sc, hc, :], wT_sb[:, sc, :],
                                 start=(sc == 0), stop=(sc == 1))
            nc.vector.tensor_copy(ctxT_sb[:, hc, :], ps[:])
        lps = ctps.tile([C, 128], F32, tag="log")
        for kc in range(KC_H2):
            nc.tensor.matmul(lps[:], clsw_sb[:, kc, :], ctxT_sb[:, kc, :],
                             start=(kc == 0), stop=(kc == KC_H2 - 1))
        lsb = ap2b.tile([C, 128], F32)
        nc.vector.tensor_scalar_add(out=lsb[:], in0=lps[:], scalar1=clsb_sb[:])
        nc.sync.dma_start(out=d["out"][:], in_=lsb[:])


# ---------------- host side ----------------

def _prep_inputs(inputs):
    """Per-core input maps from the full problem inputs."""
    ids = np.asarray(inputs["input_ids"])
    emb = np.asarray(inputs["emb"], np.float32)
    w_ih0 = np.asarray(inputs["w_ih0"], np.float32)[:, _PERM, :].copy()
    w_hh0 = np.asarray(inputs["w_hh0"], np.float32)[:, _PERM, :].copy()
    b0 = np.asarray(inputs["b0"], np.float32)[:, _PERM].copy()
    w_ih = np.asarray(inputs["w_ih"], np.float32)[:, :, _PERM, :].copy()
    w_hh = np.asarray(inputs["w_hh"], np.float32)[:, :, _PERM, :].copy()
    b = np.asarray(inputs["b"], np.float32)[:, :, _PERM].copy()
    # tanh-as-sigmoid identity: scale g-gate rows x2
    w_ih0[:, 768:] *= 2.0
    w_hh0[:, 768:] *= 2.0
    b0[:, 768:] *= 2.0
    w_ih[:, :, 768:] *= 2.0
    w_hh[:, :, 768:] *= 2.0
    b[:, :, 768:] *= 2.0
    attn_W = np.asarray(inputs["attn_W"], np.float32)
    attn_U = np.asarray(inputs["attn_U"], np.float32)
    attn_v = np.asarray(inputs["attn_v"], np.float32)
    cls_W = np.asarray(inputs["cls_W"], np.float32)
    cls_b = np.asarray(inputs["cls_b"], np.float32)

    wih0T = np.empty((128, 2, MC, 128), np.float32)
    for dd in range(2):
        wih0T[:, dd] = w_ih0[dd].T.reshape(E, MC, 128)
    wihT = np.empty((128, 2, 2, KC_H2, MC, 128), np.float32)
    for li in range(2):
        for dd in range(2):
            wihT[:, li, dd] = (w_ih[li, dd].T.reshape(KC_H2, 128, MC, 128)
                               .transpose(1, 0, 2, 3))
    # whh as fp8 DoubleRow tiles: [k_part, layer, dir, mc, kc, m]
    whh8 = np.empty((128, NL, 2, MC, KC_H, 128), np.float32)
    for layer in range(NL):
        for dd in range(2):
            wt = (w_hh0[dd] if layer == 0 else w_hh[layer - 1, dd]).T
            whh8[:, layer, dd] = (wt.reshape(KC_H, 128, MC, 128)
                                  .transpose(1, 2, 0, 3))
    # bias as selector-matmul lhsT: [2, layer, dir, mc-pair, 128]
    biasT = np.empty((2, NL, 2, MC // 2, 128), np.float32)
    for layer in range(NL):
        for dd in range(2):
            bb = b0[dd] if layer == 0 else b[layer - 1, dd]
            biasT[:, layer, dd] = bb.reshape(MC // 2, 2, 128).transpose(1, 0, 2)
    ones2 = np.zeros((2, 2, 256), np.float16)
    ones2[0, 0, :] = 1.0
    ones2[1, 1, :] = 1.0
    attnT = np.empty((128, 2, KC_H2, KC_H2, 128), np.float32)
    for i, m in enumerate((attn_W, attn_U)):
        attnT[:, i] = (m.T.reshape(KC_H2, 128, KC_H2, 128)
                       .transpose(1, 0, 2, 3))
    vT = attn_v.reshape(KC_H2, 128).T.astype(np.float16)
    vsel = np.zeros((128, KC_H2, 32, 32), np.float16)
    for tj in range(32):
        vsel[:, :, tj, tj] = vT

    clsWT = cls_W.T.reshape(KC_H2, 128, C).transpose(1, 0, 2).astype(np.float16)
    clsb = cls_b.reshape(C, 1).astype(np.float32)
    id16 = np.eye(128, dtype=np.float16)
    id32 = np.eye(128, dtype=np.float32)

    # h is stored as h/2 on device; double every matrix whose input is h
    wihT *= 2.0
    whh8 *= 2.0
    attnT *= 2.0
    clsWT *= 2.0
    # gate pre-activations carry x16 so fp8 whh stays in normal range;
    # sigmoid applies 1/16
    wih0T *= GSC
    wihT *= GSC
    whh8 *= GSC
    biasT *= GSC
    common = dict(
        embT=emb.astype(np.float16),
        wih0T=wih0T.astype(np.float16),
        wihT=wihT.astype(np.float16),
        whh8=whh8.astype(ml_dtypes.float8_e4m3),
        biasT=biasT.astype(np.float16),
        ones2=ones2,
        attnT=attnT.astype(np.float16),
        vT=vT, vsel=vsel, clsWT=clsWT, clsb=clsb,
        id16=id16, id32=id32,
    )
    in_maps = []
    for c in range(N_CORES):
        bb, th = c // 2, c % 2
        sel = np.zeros((S, 128), np.float16)
        sel[np.arange(128) + th * 128, np.arange(128)] = 1.0
        m = dict(common)
        m["ids"] = ids[bb].astype(np.float32).reshape(1, S)
        m["sel"] = sel.reshape(2, 128, 128).transpose(1, 0, 2).copy()
        in_maps.append(m)
    return in_maps


_NC_CACHE = {}


def _get_nc():
    if "nc" not in _NC_CACHE:
        _NC_CACHE["nc"] = _build_nc()
    return _NC_CACHE["nc"]


def kernel(**inputs) -> np.ndarray:
    from concourse.bass_utils import run_bass_kernel_spmd

    nc = _get_nc()
    in_maps = _prep_inputs(inputs)
    res = run_bass_kernel_spmd(nc, in_maps, list(range(N_CORES)))
    out = np.empty((B, S, C), np.float32)
    for c in range(N_CORES):
        bb, th = c // 2, c % 2
        out[bb, th * 128:(th + 1) * 128, :] = res.results[c]["logitsT"].T
    return out


# revision 17
# speedup vs baseline: 1.3134x; 1.3134x over previous
"""BiLSTM diacritizer Trainium2 kernel.

8 NeuronCores, SPMD, identical program, zero collectives.
Core c -> batch row b=c//2, attention t-half th=c%2.
Each core computes its batch row's full 3-layer BiLSTM (fwd+bwd chains
interleaved), then Bahdanau attention + classifier for its 128 query
positions. Host pre-permutes/casts weights and assembles the output.

Recurrence design: gate pre-activations gx for a whole layer are
accumulated directly in PSUM (all 8 banks: [128, 2dir, 8mc, 256t] f32),
bias injected via a tiny selector matmul, and the per-step h@Whh GEMVs
run as fp8e4 DoubleRow matmuls (K=256 contracted per instruction, so 8
matmuls per dir-step instead of 17). h is carried in fp8 for the
recurrence and bulk-cast to f16 once per layer for the next layer's
input GEMM and the attention block. Weights/bias are pre-scaled x16 on
the host so fp8 stays in the normal range; the gate sigmoid applies
scale=1/16.
"""

import sys

sys.path.insert(0, "/opt/trn_rl_repo")

from contextlib import ExitStack

import numpy as np
import ml_dtypes

import concourse.bacc as bacc
import concourse.bass as bass
import concourse.tile as tile
from concourse import mybir

# Model dims (hardcoded per problem spec)
V, E, H, C = 64, 128, 256, 15
H2 = 2 * H          # 512
G = 4 * H           # 1024 gate width
B, S = 4, 256
N_CORES = 8
NL = 3              # LSTM layers
MC = G // 128       # 8 gate-dim chunks
KC_H = H // 128     # 2 h-dim chunks
KC_H2 = H2 // 128   # 4 chunks of the 512-dim layer input / hidden concat
GSC = 16.0          # gate pre-activation scale (wih/whh/bias x16 on host)

F32 = mybir.dt.float32
F16 = mybir.dt.float16
F8 = mybir.dt.float8e4
AF = mybir.ActivationFunctionType
OP = mybir.AluOpType
DR = mybir.MatmulPerfMode.DoubleRow

# Gate permutation: torch order i,f,g,o -> device order i,f,o,g
# (so sigmoid covers contiguous chunks 0..5, tanh chunks 6..7)
_PERM = np.concatenate([
    np.arange(0, 256), np.arange(256, 512), np.arange(768, 1024),
    np.arange(512, 768),
])


def _build_nc(nl=NL, s_steps=S):
    """Build the SPMD program. nl/s_steps shrinkable for fast testing."""
    nc = bacc.Bacc(None, target_bir_lowering=False, num_devices=N_CORES)

    # ---- external inputs (per-core data, same names everywhere) ----
    d = {}
    d["ids"] = nc.dram_tensor("ids", [1, S], F32, kind="ExternalInput")
    d["sel"] = nc.dram_tensor("sel", [128, 2, 128], F16, kind="ExternalInput")
    d["embT"] = nc.dram_tensor("embT", [V, E], F16, kind="ExternalInput")
    d["wih0T"] = nc.dram_tensor("wih0T", [128, 2, MC, 128], F16,
                                kind="ExternalInput")
    d["wihT"] = nc.dram_tensor("wihT", [128, 2, 2, KC_H2, MC, 128], F16,
                               kind="ExternalInput")
    d["whhT"] = nc.dram_tensor("whhT", [128, NL, 2, KC_H, MC, 128], F16,
                               kind="ExternalInput")
    d["biasT"] = nc.dram_tensor("biasT", [2, NL, 2, MC // 2, 128], F16,
                                kind="ExternalInput")
    d["attnT"] = nc.dram_tensor("attnT", [128, 2, KC_H2, KC_H2, 128], F16,
                                kind="ExternalInput")
    d["vT"] = nc.dram_tensor("vT", [128, KC_H2], F16, kind="ExternalInput")
    d["vsel"] = nc.dram_tensor("vsel", [128, KC_H2, 32, 32], F16,
                               kind="ExternalInput")
    d["clsWT"] = nc.dram_tensor("clsWT", [128, KC_H2, C], F16,
                                kind="ExternalInput")
    d["clsb"] = nc.dram_tensor("clsb", [C, 1], F32, kind="ExternalInput")
    d["id16"] = nc.dram_tensor("id16", [128, 128], F16, kind="ExternalInput")
    d["id32"] = nc.dram_tensor("id32", [128, 128], F32, kind="ExternalInput")
    d["ones2"] = nc.dram_tensor("ones2", [2, 2, 256], F16,
                                kind="ExternalInput")
    d["out"] = nc.dram_tensor("logitsT", [C, 128], F32, kind="ExternalOutput")

    with tile.TileContext(nc) as tc, ExitStack() as ctx:
        _emit(ctx, tc, nc, nl, s_steps, d)
    nc.compile()
    return nc


def _emit(ctx, tc, nc, nl, SS, d):
    fp = ctx.enter_context(tc.tile_pool(name="persist", bufs=1))

    # ---- load constants to SBUF, ordered so layer 0 can start ASAP ----
    def _alloc(name, shape, dtype):
        return fp.tile(shape, dtype, name=f"sb_{name}", tag=f"sb_{name}")

    def _dma(t, name, sl=None):
        if sl is None:
            nc.gpsimd.dma_start(out=t[:], in_=d[name][:])
        else:
            nc.gpsimd.dma_start(out=t[:, sl], in_=d[name][:, sl])

    emb_sb = _alloc("embT", [V, E], F16)
    wih0_sb = _alloc("wih0T", [128, 2, MC, 128], F16)
    whh_sb = _alloc("whhT", [128, NL, 2, KC_H, MC, 128], F16)
    bias_sb = _alloc("biasT", [2, NL, 2, MC // 2, 128], F16)
    ones2_sb = _alloc("ones2", [2, 2, 256], F16)
    wih_sb = _alloc("wihT", [128, 2, 2, KC_H2, MC, 128], F16)
    attn_sb = _alloc("attnT", [128, 2, KC_H2, KC_H2, 128], F16)
    v_sb = _alloc("vT", [128, KC_H2], F16)
    vsel_sb = _alloc("vsel", [128, KC_H2, 32, 32], F16)
    clsw_sb = _alloc("clsWT", [128, KC_H2, C], F16)
    clsb_sb = _alloc("clsb", [C, 1], F32)
    id16_sb = _alloc("id16", [128, 128], F16)
    id32_sb = _alloc("id32", [128, 128], F32)
    sel_sb = _alloc("sel", [128, 2, 128], F16)

    # early: everything layer 0 needs
    _dma(emb_sb, "embT")
    _dma(wih0_sb, "wih0T")
    _dma(bias_sb, "biasT")
    _dma(ones2_sb, "ones2")
    _dma(whh_sb, "whhT", 0)
    _dma(whh_sb, "whhT", 1)
    _dma(wih_sb, "wihT", 0)
    _dma(whh_sb, "whhT", 2)
    _dma(wih_sb, "wihT", 1)
    # late: attention/classifier-phase tensors
    _dma(id16_sb, "id16")
    _dma(attn_sb, "attnT")
    _dma(v_sb, "vT")
    _dma(vsel_sb, "vsel")
    _dma(clsw_sb, "clsWT")
    _dma(clsb_sb, "clsb")
    _dma(id32_sb, "id32")
    _dma(sel_sb, "sel")

    zeros16 = fp.tile([128, KC_H], F16)
    nc.vector.memset(zeros16[:], 0.0)

    # ---- embedding: one-hot matmul -> xT [E=128, S] f16 ----
    ids_ap = d["ids"].ap()
    ids_b = fp.tile([V, S], F32)
    nc.gpsimd.dma_start(
        out=ids_b[:],
        in_=bass.AP(tensor=ids_ap.tensor, offset=ids_ap.offset,
                    ap=[[0, V], [1, S]]),
    )
    iota_i = fp.tile([V, 1], mybir.dt.int32)
    nc.gpsimd.iota(iota_i[:], pattern=[[0, 1]], base=0, channel_multiplier=1)
    iota_f = fp.tile([V, 1], F32)
    nc.vector.tensor_copy(iota_f[:], iota_i[:])
    oh = fp.tile([V, S], F16)
    nc.vector.tensor_scalar(
        out=oh[:], in0=ids_b[:], scalar1=iota_f[:], scalar2=None,
        op0=OP.is_equal,
    )
    xT_sb = fp.tile([128, 1, S], F16)   # layer-0 input, 1 k-chunk
    with tc.tile_pool(name="embp", bufs=1, space="PSUM") as embp:
        x_ps = embp.tile([128, S], F32)
        nc.tensor.matmul(x_ps[:], emb_sb[:], oh[:], start=True, stop=True)
        nc.vector.tensor_copy(xT_sb[:, 0, :], x_ps[:])

    # ---- LSTM layers ----
    h16_pool = ctx.enter_context(tc.tile_pool(name="h16", bufs=2))
    prev = xT_sb          # [128, kc_in, S] f16
    kc_in = 1
    for layer in range(nl):
        wl = wih0_sb if layer == 0 else wih_sb
        hT16 = h16_pool.tile([128, 4, S], F16, tag="hT16")
        with (
            tc.tile_pool(name=f"gps{layer}", bufs=1, space="PSUM") as gps,
            tc.tile_pool(name=f"rsb{layer}", bufs=4) as rsb,
        ):
            # gate pre-activations, whole layer, in PSUM (all 8 banks)
            g_ps = gps.tile([128, 2, MC, S], F32)
            # bias via selector matmul: one per (dir, mc-pair) -> 1 bank
            for dd in (0, 1):
                for p in range(MC // 2):
                    nc.tensor.matmul(
                        g_ps[:, dd, 2 * p:2 * p + 2, :],
                        bias_sb[:, layer, dd, p, :], ones2_sb[:],
                        start=True, stop=True, skip_group_check=True,
                    )
            # input GEMM accumulates on top
            for dd in (0, 1):
                for mc in range(MC):
                    for kc in range(kc_in):
                        if layer == 0:
                            lhsT = wl[:, dd, mc, :]
                        else:
                            lhsT = wl[:, layer - 1, dd, kc, mc, :]
                        nc.tensor.matmul(
                            g_ps[:, dd, mc, :], lhsT, prev[:, kc, :],
                            start=False, stop=(kc == kc_in - 1),
                            skip_group_check=True,
                        )
            # recurrence: fwd (d=0) + bwd (d=1) interleaved chains.
            combo = [None, None]
            for dd in (0, 1):
                c0 = rsb.tile([128, 4], F32, tag=f"combo{dd}")
                nc.vector.memset(c0[:], 0.0)
                combo[dd] = c0
            for t in range(SS):
                for dd in (0, 1):
                    tg = t if dd == 0 else SS - 1 - t
                    tprev = tg + 1 if dd == 1 else tg - 1
                    for mc in range(MC):
                        for kc in range(KC_H):
                            if t == 0:
                                rhs = zeros16[:, kc:kc + 1]
                            else:
                                rhs = hT16[:, 2 * dd + kc, tprev:tprev + 1]
                            nc.tensor.matmul(
                                g_ps[:, dd, mc, tg:tg + 1],
                                whh_sb[:, layer, dd, kc, mc, :], rhs,
                                start=False,
                                stop=(mc == MC - 1 and kc == KC_H - 1),
                                skip_group_check=True,
                            )
                    # gates: i=0:2 f=2:4 o=4:6 g=6:8 (chunk cols).
                    # g-rows pre-scaled x2 host-side: tanh(g)=2*sig(2g)-1,
                    # so one sigmoid covers all gates. All gate weights
                    # carry x16; undo via the activation scale.
                    s_ifo = rsb.tile([128, 8], F32, tag=f"s{dd}")
                    nc.scalar.activation(s_ifo[:], g_ps[:, dd, :, tg],
                                         AF.Sigmoid, scale=1.0 / GSC)
                    cmb = combo[dd]
                    nc.vector.tensor_scalar(
                        out=cmb[:, 0:2], in0=s_ifo[:, 6:8], scalar1=2.0,
                        scalar2=-1.0, op0=OP.mult, op1=OP.add)
                    prods = rsb.tile([128, 4], F32, tag=f"p{dd}")
                    nc.vector.tensor_mul(prods[:], s_ifo[:, 0:4], cmb[:])
                    cmb_n = rsb.tile([128, 4], F32, tag=f"combo{dd}")
                    nc.vector.tensor_add(cmb_n[:, 2:4], prods[:, 0:2],
                                         prods[:, 2:4])
                    combo[dd] = cmb_n
                    # tanh(c)=2*sig(2c)-1 via free ACT scale; emit h/2 =
                    # (sig-0.5)*o; all h consumers are doubled host-side.
                    tc_t = rsb.tile([128, 2], F32, tag=f"tc{dd}")
                    nc.scalar.activation(tc_t[:], cmb_n[:, 2:4], AF.Sigmoid,
                                         scale=2.0)
                    nc.vector.scalar_tensor_tensor(
                        out=hT16[:, 2 * dd:2 * dd + 2, tg], in0=tc_t[:],
                        scalar=0.5, in1=s_ifo[:, 4:6],
                        op0=OP.subtract, op1=OP.mult)
        prev = hT16
        kc_in = KC_H2

    # ---- attention + classifier ----
    hT = prev  # [128, 4, S] f16 final hidden (transposed layout)
    ap1 = ctx.enter_context(tc.tile_pool(name="attn1", bufs=1))

    # h layout [s, h]: PE-transpose hT blocks -> h_sb[:, sc, hc, :]
    h_sb = ap1.tile([128, 2, KC_H2, 128], F16)
    with tc.tile_pool(name="trps", bufs=4, space="PSUM") as trps:
        for hc in range(KC_H2):
            for sc in range(2):
                tp = trps.tile([128, 128], F16, tag="tp")
                nc.tensor.transpose(tp[:], hT[:, hc, sc * 128:(sc + 1) * 128],
                                    id16_sb[:])
                nc.vector.tensor_copy(h_sb[:, sc, hc, :], tp[:])

    # hT_sel [h, tl] = h-cols for my t-half, via Sel matmul; then q, k
    hsel_sb = ap1.tile([128, KC_H2, 128], F16)
    qT_sb = ap1.tile([128, KC_H2, 128], F32)
    with ExitStack() as actx:
        kTp = actx.enter_context(tc.tile_pool(name="kTps", bufs=1, space="PSUM"))
        kT_ps = kTp.tile([128, KC_H2, S], F32)
        with tc.tile_pool(name="qkps", bufs=2, space="PSUM") as qkps:
            for hc in range(KC_H2):
                ps = qkps.tile([128, 128], F32, tag="sel")
                for sc in range(2):
                    nc.tensor.matmul(ps[:], h_sb[:, sc, hc, :],
                                     sel_sb[:, sc, :],
                                     start=(sc == 0), stop=(sc == 1))
                nc.vector.tensor_copy(hsel_sb[:, hc, :], ps[:])
            for mc in range(KC_H2):
                psq = qkps.tile([128, 128], F32, tag="q")
                for kc in range(KC_H2):
                    nc.tensor.matmul(psq[:], attn_sb[:, 0, kc, mc, :],
                                     hsel_sb[:, kc, :],
                                     start=(kc == 0), stop=(kc == KC_H2 - 1))
                nc.vector.tensor_copy(qT_sb[:, mc, :], psq[:])
            for mc in range(KC_H2):
                for kc in range(KC_H2):
                    nc.tensor.matmul(kT_ps[:, mc, :], attn_sb[:, 1, kc, mc, :],
                                     hT[:, kc, :],
                                     start=(kc == 0), stop=(kc == KC_H2 - 1))

        # scores[t, s] = sum_hc vT . tanh(kT + q[t]); 32 t-rows per psum
        # tile via v-selector lhsT (v in col t%32, zero rows accumulate 0)
        scp = actx.enter_context(tc.tile_pool(name="scps", bufs=2, space="PSUM"))
        scores_sb = ap1.tile([128, S], F32)
        with tc.tile_pool(name="tanhp", bufs=4) as tanhp:
            for tg_i in range(4):
                sc_ps = scp.tile([32, S], F32, tag="sc")
                for tj in range(32):
                    t = tg_i * 32 + tj
                    for hc in range(KC_H2):
                        th_t = tanhp.tile([128, S], F16, tag="th")
                        nc.scalar.activation(th_t[:], kT_ps[:, hc, :], AF.Tanh,
                                             bias=qT_sb[:, hc, t:t + 1])
                        nc.tensor.matmul(
                            sc_ps[:], vsel_sb[:, hc, tj, :], th_t[:],
                            start=(tj == 0 and hc == 0),
                            stop=(tj == 31 and hc == KC_H2 - 1),
                            skip_group_check=True,
                        )
                nc.vector.tensor_copy(
                    scores_sb[tg_i * 32:(tg_i + 1) * 32, :], sc_ps[:])

        # softmax rows -> wn_sb [128, S] f32
        ap2 = ctx.enter_context(tc.tile_pool(name="attn2", bufs=1))
        wn_sb = ap2.tile([128, S], F32)
        nmax = ap2.tile([128, 1], F32)
        nc.vector.tensor_reduce(out=nmax[:], in_=scores_sb[:], op=OP.max,
                                axis=mybir.AxisListType.X, negate=True)
        rsum = ap2.tile([128, 1], F32)
        wexp = ap2.tile([128, S], F32)
        nc.scalar.activation(wexp[:], scores_sb[:], AF.Exp,
                             bias=nmax[:], accum_out=rsum[:])
        rinv = ap2.tile([128, 1], F32)
        nc.vector.reciprocal(rinv[:], rsum[:])
        nc.vector.tensor_scalar_mul(wn_sb[:], wexp[:], rinv[:])

    # wT via PE transpose -> f16; ctxT; classifier
    ap2b = ctx.enter_context(tc.tile_pool(name="attn3", bufs=1))
    wT_sb = ap2b.tile([128, 2, 128], F16)
    ctxT_sb = ap2b.tile([128, KC_H2, 128], F16)
    with tc.tile_pool(name="ctps", bufs=2, space="PSUM") as ctps:
        for sc in range(2):
            tp32 = ctps.tile([128, 128], F32, tag="wt")
            nc.tensor.transpose(tp32[:], wn_sb[:, sc * 128:(sc + 1) * 128],
                                id32_sb[:])
            nc.vector.tensor_copy(wT_sb[:, sc, :], tp32[:])
        for hc in range(KC_H2):
            ps = ctps.tile([128, 128], F32, tag="ctx")
            for sc in range(2):
                nc.tensor.matmul(ps[:], h_sb[:, sc, hc, :], wT_sb[:, sc, :],
                                 start=(sc == 0), stop=(sc == 1))
            nc.vector.tensor_copy(ctxT_sb[:, hc, :], ps[:])
        lps = ctps.tile([C, 128], F32, tag="log")
        for kc in range(KC_H2):
            nc.tensor.matmul(lps[:], clsw_sb[:, kc, :], ctxT_sb[:, kc, :],
                             start=(kc == 0), stop=(kc == KC_H2 - 1))
        lsb = ap2b.tile([C, 128], F32)
        nc.vector.tensor_scalar_add(out=lsb[:], in0=lps[:], scalar1=clsb_sb[:])
        nc.sync.dma_start(out=d["out"][:], in_=lsb[:])


# ---------------- host side ----------------

def _prep_inputs(inputs):
    """Per-core input maps from the full problem inputs."""
    ids = np.asarray(inputs["input_ids"])
    emb = np.asarray(inputs["emb"], np.float32)
    w_ih0 = np.asarray(inputs["w_ih0"], np.float32)[:, _PERM, :].copy()
    w_hh0 = np.asarray(inputs["w_hh0"], np.float32)[:, _PERM, :].copy()
    b0 = np.asarray(inputs["b0"], np.float32)[:, _PERM].copy()
    w_ih = np.asarray(inputs["w_ih"], np.float32)[:, :, _PERM, :].copy()
    w_hh = np.asarray(inputs["w_hh"], np.float32)[:, :, _PERM, :].copy()
    b = np.asarray(inputs["b"], np.float32)[:, :, _PERM].copy()
    # tanh-as-sigmoid identity: scale g-gate rows x2
    w_ih0[:, 768:] *= 2.0
    w_hh0[:, 768:] *= 2.0
    b0[:, 768:] *= 2.0
    w_ih[:, :, 768:] *= 2.0
    w_hh[:, :, 768:] *= 2.0
    b[:, :, 768:] *= 2.0
    attn_W = np.asarray(inputs["attn_W"], np.float32)
    attn_U = np.asarray(inputs["attn_U"], np.float32)
    attn_v = np.asarray(inputs["attn_v"], np.float32)
    cls_W = np.asarray(inputs["cls_W"], np.float32)
    cls_b = np.asarray(inputs["cls_b"], np.float32)

    wih0T = np.empty((128, 2, MC, 128), np.float32)
    for dd in range(2):
        wih0T[:, dd] = w_ih0[dd].T.reshape(E, MC, 128)
    wihT = np.empty((128, 2, 2, KC_H2, MC, 128), np.float32)
    for li in range(2):
        for dd in range(2):
            wihT[:, li, dd] = (w_ih[li, dd].T.reshape(KC_H2, 128, MC, 128)
                               .transpose(1, 0, 2, 3))
    whhT = np.empty((128, NL, 2, KC_H, MC, 128), np.float32)
    for layer in range(NL):
        for dd in range(2):
            wt = (w_hh0[dd] if layer == 0 else w_hh[layer - 1, dd]).T
            whhT[:, layer, dd] = (wt.reshape(KC_H, 128, MC, 128)
                                  .transpose(1, 0, 2, 3))
    # bias as selector-matmul lhsT: [2, layer, dir, mc-pair, 128]
    biasT = np.empty((2, NL, 2, MC // 2, 128), np.float32)
    for layer in range(NL):
        for dd in range(2):
            bb = b0[dd] if layer == 0 else b[layer - 1, dd]
            biasT[:, layer, dd] = bb.reshape(MC // 2, 2, 128).transpose(1, 0, 2)
    ones2 = np.zeros((2, 2, 256), np.float16)
    ones2[0, 0, :] = 1.0
    ones2[1, 1, :] = 1.0
    attnT = np.empty((128, 2, KC_H2, KC_H2, 128), np.float32)
    for i, m in enumerate((attn_W, attn_U)):
        attnT[:, i] = (m.T.reshape(KC_H2, 128, KC_H2, 128)
                       .transpose(1, 0, 2, 3))
    vT = attn_v.reshape(KC_H2, 128).T.astype(np.float16)
    vsel = np.zeros((128, KC_H2, 32, 32), np.float16)
    for tj in range(32):
        vsel[:, :, tj, tj] = vT

    clsWT = cls_W.T.reshape(KC_H2, 128, C).transpose(1, 0, 2).astype(np.float16)
    clsb = cls_b.reshape(C, 1).astype(np.float32)
    id16 = np.eye(128, dtype=np.float16)
    id32 = np.eye(128, dtype=np.float32)

    # h is stored as h/2 on device; double every matrix whose input is h
    wihT *= 2.0
    whhT *= 2.0
    attnT *= 2.0
    clsWT *= 2.0
    # gate pre-activations carry x16 (kept from the fp8 experiments, the
    # sigmoid applies 1/16; harmless in f16 since |g*16| < 512)
    wih0T *= GSC
    wihT *= GSC
    whhT *= GSC
    biasT *= GSC
    common = dict(
        embT=emb.astype(np.float16),
        wih0T=wih0T.astype(np.float16),
        wihT=wihT.astype(np.float16),
        whhT=whhT.astype(np.float16),
        biasT=biasT.astype(np.float16),
        ones2=ones2,
        attnT=attnT.astype(np.float16),
        vT=vT, vsel=vsel, clsWT=clsWT, clsb=clsb,
        id16=id16, id32=id32,
    )
    in_maps = []
    for c in range(N_CORES):
        bb, th = c // 2, c % 2
        sel = np.zeros((S, 128), np.float16)
        sel[np.arange(128) + th * 128, np.arange(128)] = 1.0
        m = dict(common)
        m["ids"] = ids[bb].astype(np.float32).reshape(1, S)
        m["sel"] = sel.reshape(2, 128, 128).transpose(1, 0, 2).copy()
        in_maps.append(m)
    return in_maps


_NC_CACHE = {}


def _get_nc():
    if "nc" not in _NC_CACHE:
        _NC_CACHE["nc"] = _build_nc()
    return _NC_CACHE["nc"]


def kernel(**inputs) -> np.ndarray:
    from concourse.bass_utils import run_bass_kernel_spmd

    nc = _get_nc()
    in_maps = _prep_inputs(inputs)
    res = run_bass_kernel_spmd(nc, in_maps, list(range(N_CORES)))
    out = np.empty((B, S, C), np.float32)
    for c in range(N_CORES):
        bb, th = c // 2, c % 2
        out[bb, th * 128:(th + 1) * 128, :] = res.results[c]["logitsT"].T
    return out


# revision 23
# speedup vs baseline: 1.4832x; 1.1293x over previous
"""BiLSTM diacritizer Trainium2 kernel.

8 NeuronCores, SPMD, identical program, zero collectives.
Core c -> batch row b=c//2, attention t-half th=c%2.
Each core computes its batch row's full 3-layer BiLSTM (fwd+bwd chains
interleaved), then Bahdanau attention + classifier for its 128 query
positions. Host pre-permutes/casts weights and assembles the output.

Recurrence design: gate pre-activations gx for a whole layer are
accumulated directly in PSUM (all 8 banks: [128, 2dir, 8mc, 256t] f32),
bias injected via a tiny selector matmul, and the per-step h@Whh GEMVs
run as fp8e4 DoubleRow matmuls (K=256 contracted per instruction, so 8
matmuls per dir-step instead of 17). h is carried in fp8 for the
recurrence and bulk-cast to f16 once per layer for the next layer's
input GEMM and the attention block. Weights/bias are pre-scaled x16 on
the host so fp8 stays in the normal range; the gate sigmoid applies
scale=1/16.
"""

import sys

sys.path.insert(0, "/opt/trn_rl_repo")

from contextlib import ExitStack

import numpy as np
import ml_dtypes

import concourse.bacc as bacc
import concourse.bass as bass
import concourse.tile as tile
from concourse import mybir

# Model dims (hardcoded per problem spec)
V, E, H, C = 64, 128, 256, 15
H2 = 2 * H          # 512
G = 4 * H           # 1024 gate width
B, S = 4, 256
N_CORES = 8
NL = 3              # LSTM layers
MC = G // 128       # 8 gate-dim chunks
KC_H = H // 128     # 2 h-dim chunks
KC_H2 = H2 // 128   # 4 chunks of the 512-dim layer input / hidden concat
GSC = 16.0          # gate pre-activation scale (wih/whh/bias x16 on host)

F32 = mybir.dt.float32
F16 = mybir.dt.float16
F8 = mybir.dt.float8e4
AF = mybir.ActivationFunctionType
OP = mybir.AluOpType
DR = mybir.MatmulPerfMode.DoubleRow

# Gate permutation: torch order i,f,g,o -> device order i,f,o,g
# (so sigmoid covers contiguous chunks 0..5, tanh chunks 6..7)
_PERM = np.concatenate([
    np.arange(0, 256), np.arange(256, 512), np.arange(768, 1024),
    np.arange(512, 768),
])


def _build_nc(nl=NL, s_steps=S):
    """Build the SPMD program. nl/s_steps shrinkable for fast testing."""
    nc = bacc.Bacc(None, target_bir_lowering=False, num_devices=N_CORES)

    # ---- external inputs (per-core data, same names everywhere) ----
    d = {}
    d["ids"] = nc.dram_tensor("ids", [1, S], F32, kind="ExternalInput")
    d["sel"] = nc.dram_tensor("sel", [128, 2, 128], F16, kind="ExternalInput")
    d["embT"] = nc.dram_tensor("embT", [V, E], F16, kind="ExternalInput")
    d["wih0T"] = nc.dram_tensor("wih0T", [128, 2, MC, 128], F16,
                                kind="ExternalInput")
    d["wihT"] = nc.dram_tensor("wihT", [128, 2, 2, KC_H2, MC, 128], F16,
                               kind="ExternalInput")
    d["whhT"] = nc.dram_tensor("whhT", [128, NL, 2, KC_H, MC, 128], F16,
                               kind="ExternalInput")
    d["biasT"] = nc.dram_tensor("biasT", [2, NL, 2, MC // 2, 128], F16,
                                kind="ExternalInput")
    d["attnT"] = nc.dram_tensor("attnT", [128, 2, KC_H2, KC_H2, 128], F16,
                                kind="ExternalInput")
    d["vT"] = nc.dram_tensor("vT", [128, KC_H2], F16, kind="ExternalInput")
    d["vsel"] = nc.dram_tensor("vsel", [128, KC_H2, 32, 32], F16,
                               kind="ExternalInput")
    d["clsWT"] = nc.dram_tensor("clsWT", [128, KC_H2, C], F16,
                                kind="ExternalInput")
    d["clsb"] = nc.dram_tensor("clsb", [C, 1], F32, kind="ExternalInput")
    d["id16"] = nc.dram_tensor("id16", [128, 128], F16, kind="ExternalInput")
    d["id32"] = nc.dram_tensor("id32", [128, 128], F32, kind="ExternalInput")
    d["ones2"] = nc.dram_tensor("ones2", [2, 2, 256], F16,
                                kind="ExternalInput")
    d["out"] = nc.dram_tensor("logitsT", [C, 128], F32, kind="ExternalOutput")

    with tile.TileContext(nc) as tc, ExitStack() as ctx:
        _emit(ctx, tc, nc, nl, s_steps, d)
    nc.compile()
    return nc


def _emit(ctx, tc, nc, nl, SS, d):
    fp = ctx.enter_context(tc.tile_pool(name="persist", bufs=1))

    # ---- load constants to SBUF, ordered so layer 0 can start ASAP ----
    def _alloc(name, shape, dtype):
        return fp.tile(shape, dtype, name=f"sb_{name}", tag=f"sb_{name}")

    def _dma(t, name, sl=None):
        if sl is None:
            nc.gpsimd.dma_start(out=t[:], in_=d[name][:])
        else:
            nc.gpsimd.dma_start(out=t[:, sl], in_=d[name][:, sl])

    emb_sb = _alloc("embT", [V, E], F16)
    wih0_sb = _alloc("wih0T", [128, 2, MC, 128], F16)
    whh_sb = _alloc("whhT", [128, NL, 2, KC_H, MC, 128], F16)
    bias_sb = _alloc("biasT", [2, NL, 2, MC // 2, 128], F16)
    ones2_sb = _alloc("ones2", [2, 2, 256], F16)
    wih_sb = _alloc("wihT", [128, 2, 2, KC_H2, MC, 128], F16)
    attn_sb = _alloc("attnT", [128, 2, KC_H2, KC_H2, 128], F16)
    v_sb = _alloc("vT", [128, KC_H2], F16)
    vsel_sb = _alloc("vsel", [128, KC_H2, 32, 32], F16)
    clsw_sb = _alloc("clsWT", [128, KC_H2, C], F16)
    clsb_sb = _alloc("clsb", [C, 1], F32)
    id16_sb = _alloc("id16", [128, 128], F16)
    id32_sb = _alloc("id32", [128, 128], F32)
    sel_sb = _alloc("sel", [128, 2, 128], F16)

    # early: everything layer 0 needs; ids first (embedding is the very
    # first compute)
    ids_ap = d["ids"].ap()
    ids_b = fp.tile([V, S], F32)
    nc.gpsimd.dma_start(
        out=ids_b[:],
        in_=bass.AP(tensor=ids_ap.tensor, offset=ids_ap.offset,
                    ap=[[0, V], [1, S]]),
    )
    _dma(emb_sb, "embT")
    _dma(wih0_sb, "wih0T")
    _dma(bias_sb, "biasT")
    _dma(ones2_sb, "ones2")
    _dma(whh_sb, "whhT", 0)
    _dma(whh_sb, "whhT", 1)
    _dma(wih_sb, "wihT", 0)
    _dma(whh_sb, "whhT", 2)
    _dma(wih_sb, "wihT", 1)
    # late: attention/classifier-phase tensors
    _dma(id16_sb, "id16")
    _dma(attn_sb, "attnT")
    _dma(v_sb, "vT")
    _dma(vsel_sb, "vsel")
    _dma(clsw_sb, "clsWT")
    _dma(clsb_sb, "clsb")
    _dma(id32_sb, "id32")
    _dma(sel_sb, "sel")

    zeros16 = fp.tile([128, KC_H], F16)
    nc.vector.memset(zeros16[:], 0.0)
    neg2 = fp.tile([128, 1], F32)
    nc.vector.memset(neg2[:], -2.0)

    # ---- embedding: one-hot matmul -> xT [E=128, S] f16 ----
    iota_i = fp.tile([V, 1], mybir.dt.int32)
    nc.gpsimd.iota(iota_i[:], pattern=[[0, 1]], base=0, channel_multiplier=1)
    iota_f = fp.tile([V, 1], F32)
    nc.vector.tensor_copy(iota_f[:], iota_i[:])
    oh = fp.tile([V, S], F16)
    nc.vector.tensor_scalar(
        out=oh[:], in0=ids_b[:], scalar1=iota_f[:], scalar2=None,
        op0=OP.is_equal,
    )
    xT_sb = fp.tile([128, 1, S], F16)   # layer-0 input, 1 k-chunk
    with tc.tile_pool(name="embp", bufs=1, space="PSUM") as embp:
        x_ps = embp.tile([128, S], F32)
        nc.tensor.matmul(x_ps[:], emb_sb[:], oh[:], start=True, stop=True)
        nc.vector.tensor_copy(xT_sb[:, 0, :], x_ps[:])

    # ---- LSTM layers ----
    h16_pool = ctx.enter_context(tc.tile_pool(name="h16", bufs=2))
    prev = xT_sb          # [128, kc_in, S] f16
    kc_in = 1
    for layer in range(nl):
        wl = wih0_sb if layer == 0 else wih_sb
        hT16 = h16_pool.tile([128, 4, S], F16, tag="hT16")
        with (
            tc.tile_pool(name=f"gps{layer}", bufs=1, space="PSUM") as gps,
            tc.tile_pool(name=f"rsb{layer}", bufs=4) as rsb,
        ):
            # gate pre-activations, whole layer, in PSUM (all 8 banks)
            g_ps = gps.tile([128, 2, MC, S], F32)
            # bias via selector matmul: one per (dir, mc-pair) -> 1 bank
            for dd in (0, 1):
                for p in range(MC // 2):
                    nc.tensor.matmul(
                        g_ps[:, dd, 2 * p:2 * p + 2, :],
                        bias_sb[:, layer, dd, p, :], ones2_sb[:],
                        start=True, stop=True, skip_group_check=True,
                    )
            # input GEMM accumulates on top
            for dd in (0, 1):
                for mc in range(MC):
                    for kc in range(kc_in):
                        if layer == 0:
                            lhsT = wl[:, dd, mc, :]
                        else:
                            lhsT = wl[:, layer - 1, dd, kc, mc, :]
                        nc.tensor.matmul(
                            g_ps[:, dd, mc, :], lhsT, prev[:, kc, :],
                            start=False, stop=(kc == kc_in - 1),
                            skip_group_check=True,
                        )
            # recurrence: fwd (d=0) + bwd (d=1) interleaved chains.
            # Cell state is carried as D = c/2 + 1/2 in cols 8:10 of the
            # per-step gate tile st = [sig(i) sig(f) sig(o) sig(2g) | D],
            # so ONE scalar_tensor_tensor of form (x-1/2)*y yields both
            # i*tanh(g)/2 = (sig(2g)-1/2)*sig(i) and f*c/2 = (D-1/2)*sig(f),
            # a second stt folds the +1/2 into the sum, and tanh(c) =
            # 2*sig(2c)-1 comes from sig(4D-2) via ACT scale+bias.
            # 3 DVE ops + 2 ACT ops per dir-step, 5 serial stages.
            st = [[None, None], [None, None]]
            for dd in (0, 1):
                for par in (0, 1):
                    st[dd][par] = rsb.tile([128, 10], F32,
                                           tag=f"st{dd}{par}",
                                           name=f"st{dd}{par}_{layer}")
                nc.vector.memset(st[dd][0][:, 8:10], 0.5)
            for t in range(SS):
                for dd in (0, 1):
                    tg = t if dd == 0 else SS - 1 - t
                    tprev = tg + 1 if dd == 1 else tg - 1
                    for mc in range(MC):
                        for kc in range(KC_H):
                            if t == 0:
                                rhs = zeros16[:, kc:kc + 1]
                            else:
                                rhs = hT16[:, 2 * dd + kc, tprev:tprev + 1]
                            nc.tensor.matmul(
                                g_ps[:, dd, mc, tg:tg + 1],
                                whh_sb[:, layer, dd, kc, mc, :], rhs,
                                start=False,
                                stop=(mc == MC - 1 and kc == KC_H - 1),
                                skip_group_check=True,
                            )
                    # gates: i=0:2 f=2:4 o=4:6 g=6:8 (chunk cols).
                    # g-rows pre-scaled x2 host-side; all gate weights
                    # carry x16, undone via the activation scale.
                    cur = st[dd][t % 2]
                    nxt = st[dd][(t + 1) % 2]
                    nc.scalar.activation(cur[:, 0:8], g_ps[:, dd, :, tg],
                                         AF.Sigmoid, scale=1.0 / GSC)
                    prods = rsb.tile([128, 4], F32, tag=f"p{dd}")
                    nc.vector.scalar_tensor_tensor(
                        out=prods[:], in0=cur[:, 6:10], scalar=0.5,
                        in1=cur[:, 0:4], op0=OP.subtract, op1=OP.mult)
                    nc.vector.scalar_tensor_tensor(
                        out=nxt[:, 8:10], in0=prods[:, 0:2], scalar=0.5,
                        in1=prods[:, 2:4], op0=OP.add, op1=OP.add)
                    tc_t = rsb.tile([128, 2], F32, tag=f"tc{dd}")
                    nc.scalar.activation(tc_t[:], nxt[:, 8:10], AF.Sigmoid,
                                         scale=4.0, bias=neg2[:])
                    nc.vector.scalar_tensor_tensor(
                        out=hT16[:, 2 * dd:2 * dd + 2, tg], in0=tc_t[:],
                        scalar=0.5, in1=cur[:, 4:6],
                        op0=OP.subtract, op1=OP.mult)
        prev = hT16
        kc_in = KC_H2

    # ---- attention + classifier ----
    hT = prev  # [128, 4, S] f16 final hidden (transposed layout)
    ap1 = ctx.enter_context(tc.tile_pool(name="attn1", bufs=1))

    # h layout [s, h]: PE-transpose hT blocks -> h_sb[:, sc, hc, :]
    h_sb = ap1.tile([128, 2, KC_H2, 128], F16)
    with tc.tile_pool(name="trps", bufs=4, space="PSUM") as trps:
        for hc in range(KC_H2):
            for sc in range(2):
                tp = trps.tile([128, 128], F16, tag="tp")
                nc.tensor.transpose(tp[:], hT[:, hc, sc * 128:(sc + 1) * 128],
                                    id16_sb[:])
                nc.vector.tensor_copy(h_sb[:, sc, hc, :], tp[:])

    # hT_sel [h, tl] = h-cols for my t-half, via Sel matmul; then q, k
    hsel_sb = ap1.tile([128, KC_H2, 128], F16)
    qT_sb = ap1.tile([128, KC_H2, 128], F32)
    with ExitStack() as actx:
        kTp = actx.enter_context(tc.tile_pool(name="kTps", bufs=1, space="PSUM"))
        kT_ps = kTp.tile([128, KC_H2, S], F32)
        with tc.tile_pool(name="qkps", bufs=2, space="PSUM") as qkps:
            for hc in range(KC_H2):
                ps = qkps.tile([128, 128], F32, tag="sel")
                for sc in range(2):
                    nc.tensor.matmul(ps[:], h_sb[:, sc, hc, :],
                                     sel_sb[:, sc, :],
                                     start=(sc == 0), stop=(sc == 1))
                nc.vector.tensor_copy(hsel_sb[:, hc, :], ps[:])
            for mc in range(KC_H2):
                psq = qkps.tile([128, 128], F32, tag="q")
                for kc in range(KC_H2):
                    nc.tensor.matmul(psq[:], attn_sb[:, 0, kc, mc, :],
                                     hsel_sb[:, kc, :],
                                     start=(kc == 0), stop=(kc == KC_H2 - 1))
                nc.vector.tensor_copy(qT_sb[:, mc, :], psq[:])
            for mc in range(KC_H2):
                for kc in range(KC_H2):
                    nc.tensor.matmul(kT_ps[:, mc, :], attn_sb[:, 1, kc, mc, :],
                                     hT[:, kc, :],
                                     start=(kc == 0), stop=(kc == KC_H2 - 1))

        # scores[t, s] = sum_hc vT . tanh(kT + q[t]); 32 t-rows per psum
        # tile via v-selector lhsT (v in col t%32, zero rows accumulate 0)
        scp = actx.enter_context(tc.tile_pool(name="scps", bufs=2, space="PSUM"))
        scores_sb = ap1.tile([128, S], F32)
        with tc.tile_pool(name="tanhp", bufs=4) as tanhp:
            for tg_i in range(4):
                sc_ps = scp.tile([32, S], F32, tag="sc")
                for tj in range(32):
                    t = tg_i * 32 + tj
                    for hc in range(KC_H2):
                        th_t = tanhp.tile([128, S], F16, tag="th")
                        nc.scalar.activation(th_t[:], kT_ps[:, hc, :], AF.Tanh,
                                             bias=qT_sb[:, hc, t:t + 1])
                        nc.tensor.matmul(
                            sc_ps[:], vsel_sb[:, hc, tj, :], th_t[:],
                            start=(tj == 0 and hc == 0),
                            stop=(tj == 31 and hc == KC_H2 - 1),
                            skip_group_check=True,
                        )
                nc.vector.tensor_copy(
                    scores_sb[tg_i * 32:(tg_i + 1) * 32, :], sc_ps[:])

        # softmax rows -> wn_sb [128, S] f32
        ap2 = ctx.enter_context(tc.tile_pool(name="attn2", bufs=1))
        wn_sb = ap2.tile([128, S], F32)
        nmax = ap2.tile([128, 1], F32)
        nc.vector.tensor_reduce(out=nmax[:], in_=scores_sb[:], op=OP.max,
                                axis=mybir.AxisListType.X, negate=True)
        rsum = ap2.tile([128, 1], F32)
        wexp = ap2.tile([128, S], F32)
        nc.scalar.activation(wexp[:], scores_sb[:], AF.Exp,
                             bias=nmax[:], accum_out=rsum[:])
        rinv = ap2.tile([128, 1], F32)
        nc.vector.reciprocal(rinv[:], rsum[:])
        nc.vector.tensor_scalar_mul(wn_sb[:], wexp[:], rinv[:])

    # wT via PE transpose -> f16; ctxT; classifier
    ap2b = ctx.enter_context(tc.tile_pool(name="attn3", bufs=1))
    wT_sb = ap2b.tile([128, 2, 128], F16)
    ctxT_sb = ap2b.tile([128, KC_H2, 128], F16)
    with tc.tile_pool(name="ctps", bufs=2, space="PSUM") as ctps:
        for sc in range(2):
            tp32 = ctps.tile([128, 128], F32, tag="wt")
            nc.tensor.transpose(tp32[:], wn_sb[:, sc * 128:(sc + 1) * 128],
                                id32_sb[:])
            nc.vector.tensor_copy(wT_sb[:, sc, :], tp32[:])
        for hc in range(KC_H2):
            ps = ctps.tile([128, 128], F32, tag="ctx")
            for sc in range(2):
                nc.tensor.matmul(ps[:], h_sb[:, sc, hc, :], wT_sb[:, sc, :],
                                 start=(sc == 0), stop=(sc == 1))
            nc.vector.tensor_copy(ctxT_sb[:, hc, :], ps[:])
        lps = ctps.tile([C, 128], F32, tag="log")
        for kc in range(KC_H2):
            nc.tensor.matmul(lps[:], clsw_sb[:, kc, :], ctxT_sb[:, kc, :],
                             start=(kc == 0), stop=(kc == KC_H2 - 1))
        lsb = ap2b.tile([C, 128], F32)
        nc.vector.tensor_scalar_add(out=lsb[:], in0=lps[:], scalar1=clsb_sb[:])
        nc.sync.dma_start(out=d["out"][:], in_=lsb[:])


# ---------------- host side ----------------

def _prep_inputs(inputs):
    """Per-core input maps from the full problem inputs."""
    ids = np.asarray(inputs["input_ids"])
    emb = np.asarray(inputs["emb"], np.float32)
    w_ih0 = np.asarray(inputs["w_ih0"], np.float32)[:, _PERM, :].copy()
    w_hh0 = np.asarray(inputs["w_hh0"], np.float32)[:, _PERM, :].copy()
    b0 = np.asarray(inputs["b0"], np.float32)[:, _PERM].copy()
    w_ih = np.asarray(inputs["w_ih"], np.float32)[:, :, _PERM, :].copy()
    w_hh = np.asarray(inputs["w_hh"], np.float32)[:, :, _PERM, :].copy()
    b = np.asarray(inputs["b"], np.float32)[:, :, _PERM].copy()
    # tanh-as-sigmoid identity: scale g-gate rows x2
    w_ih0[:, 768:] *= 2.0
    w_hh0[:, 768:] *= 2.0
    b0[:, 768:] *= 2.0
    w_ih[:, :, 768:] *= 2.0
    w_hh[:, :, 768:] *= 2.0
    b[:, :, 768:] *= 2.0
    attn_W = np.asarray(inputs["attn_W"], np.float32)
    attn_U = np.asarray(inputs["attn_U"], np.float32)
    attn_v = np.asarray(inputs["attn_v"], np.float32)
    cls_W = np.asarray(inputs["cls_W"], np.float32)
    cls_b = np.asarray(inputs["cls_b"], np.float32)

    wih0T = np.empty((128, 2, MC, 128), np.float32)
    for dd in range(2):
        wih0T[:, dd] = w_ih0[dd].T.reshape(E, MC, 128)
    wihT = np.empty((128, 2, 2, KC_H2, MC, 128), np.float32)
    for li in range(2):
        for dd in range(2):
            wihT[:, li, dd] = (w_ih[li, dd].T.reshape(KC_H2, 128, MC, 128)
                               .transpose(1, 0, 2, 3))
    whhT = np.empty((128, NL, 2, KC_H, MC, 128), np.float32)
    for layer in range(NL):
        for dd in range(2):
            wt = (w_hh0[dd] if layer == 0 else w_hh[layer - 1, dd]).T
            whhT[:, layer, dd] = (wt.reshape(KC_H, 128, MC, 128)
                                  .transpose(1, 0, 2, 3))
    # bias as selector-matmul lhsT: [2, layer, dir, mc-pair, 128]
    biasT = np.empty((2, NL, 2, MC // 2, 128), np.float32)
    for layer in range(NL):
        for dd in range(2):
            bb = b0[dd] if layer == 0 else b[layer - 1, dd]
            biasT[:, layer, dd] = bb.reshape(MC // 2, 2, 128).transpose(1, 0, 2)
    ones2 = np.zeros((2, 2, 256), np.float16)
    ones2[0, 0, :] = 1.0
    ones2[1, 1, :] = 1.0
    attnT = np.empty((128, 2, KC_H2, KC_H2, 128), np.float32)
    for i, m in enumerate((attn_W, attn_U)):
        attnT[:, i] = (m.T.reshape(KC_H2, 128, KC_H2, 128)
                       .transpose(1, 0, 2, 3))
    vT = attn_v.reshape(KC_H2, 128).T.astype(np.float16)
    vsel = np.zeros((128, KC_H2, 32, 32), np.float16)
    for tj in range(32):
        vsel[:, :, tj, tj] = vT

    clsWT = cls_W.T.reshape(KC_H2, 128, C).transpose(1, 0, 2).astype(np.float16)
    clsb = cls_b.reshape(C, 1).astype(np.float32)
    id16 = np.eye(128, dtype=np.float16)
    id32 = np.eye(128, dtype=np.float32)

    # h is stored as h/2 on device; double every matrix whose input is h
    wihT *= 2.0
    whhT *= 2.0
    attnT *= 2.0
    clsWT *= 2.0
    # gate pre-activations carry x16 (kept from the fp8 experiments, the
    # sigmoid applies 1/16; harmless in f16 since |g*16| < 512)
    wih0T *= GSC
    wihT *= GSC
    whhT *= GSC
    biasT *= GSC
    common = dict(
        embT=emb.astype(np.float16),
        wih0T=wih0T.astype(np.float16),
        wihT=wihT.astype(np.float16),
        whhT=whhT.astype(np.float16),
        biasT=biasT.astype(np.float16),
        ones2=ones2,
        attnT=attnT.astype(np.float16),
        vT=vT, vsel=vsel, clsWT=clsWT, clsb=clsb,
        id16=id16, id32=id32,
    )
    in_maps = []
    for c in range(N_CORES):
        bb, th = c // 2, c % 2
        sel = np.zeros((S, 128), np.float16)
        sel[np.arange(128) + th * 128, np.arange(128)] = 1.0
        m = dict(common)
        m["ids"] = ids[bb].astype(np.float32).reshape(1, S)
        m["sel"] = sel.reshape(2, 128, 128).transpose(1, 0, 2).copy()
        in_maps.append(m)
    return in_maps


_NC_CACHE = {}


def _get_nc():
    if "nc" not in _NC_CACHE:
        _NC_CACHE["nc"] = _build_nc()
    return _NC_CACHE["nc"]


def kernel(**inputs) -> np.ndarray:
    from concourse.bass_utils import run_bass_kernel_spmd

    nc = _get_nc()
    in_maps = _prep_inputs(inputs)
    res = run_bass_kernel_spmd(nc, in_maps, list(range(N_CORES)))
    out = np.empty((B, S, C), np.float32)
    for c in range(N_CORES):
        bb, th = c // 2, c % 2
        out[bb, th * 128:(th + 1) * 128, :] = res.results[c]["logitsT"].T
    return out


# revision 50
# speedup vs baseline: 1.4947x; 1.0078x over previous
"""BiLSTM diacritizer Trainium2 kernel.

8 NeuronCores, SPMD, identical program, zero collectives.
Core c -> batch row b=c//2, attention t-half th=c%2.
Each core computes its batch row's full 3-layer BiLSTM (fwd+bwd chains
interleaved), then Bahdanau attention + classifier for its 128 query
positions. Host pre-permutes/casts weights and assembles the output.

Recurrence design: gate pre-activations gx for a whole layer are
accumulated directly in PSUM (all 8 banks: [128, 2dir, 8mc, 256t] f32),
bias injected via a tiny selector matmul, and the per-step h@Whh GEMVs
run as fp8e4 DoubleRow matmuls (K=256 contracted per instruction, so 8
matmuls per dir-step instead of 17). h is carried in fp8 for the
recurrence and bulk-cast to f16 once per layer for the next layer's
input GEMM and the attention block. Weights/bias are pre-scaled x16 on
the host so fp8 stays in the normal range; the gate sigmoid applies
scale=1/16.
"""

import sys

sys.path.insert(0, "/opt/trn_rl_repo")

from contextlib import ExitStack

import numpy as np
import ml_dtypes

import concourse.bacc as bacc
import concourse.bass as bass
import concourse.tile as tile
from concourse import mybir

# Model dims (hardcoded per problem spec)
V, E, H, C = 64, 128, 256, 15
H2 = 2 * H          # 512
G = 4 * H           # 1024 gate width
B, S = 4, 256
N_CORES = 8
NL = 3              # LSTM layers
MC = G // 128       # 8 gate-dim chunks
KC_H = H // 128     # 2 h-dim chunks
KC_H2 = H2 // 128   # 4 chunks of the 512-dim layer input / hidden concat
GSC = 16.0          # gate pre-activation scale (wih/whh/bias x16 on host)

F32 = mybir.dt.float32
F16 = mybir.dt.float16
F8 = mybir.dt.float8e4
AF = mybir.ActivationFunctionType
OP = mybir.AluOpType
DR = mybir.MatmulPerfMode.DoubleRow

# Gate permutation: torch order i,f,g,o -> device order i,f,o,g
# (so sigmoid covers contiguous chunks 0..5, tanh chunks 6..7)
_PERM = np.concatenate([
    np.arange(0, 256), np.arange(256, 512), np.arange(768, 1024),
    np.arange(512, 768),
])


def _build_nc(nl=NL, s_steps=S):
    """Build the SPMD program. nl/s_steps shrinkable for fast testing."""
    nc = bacc.Bacc(None, target_bir_lowering=False, num_devices=N_CORES)

    # ---- external inputs (per-core data, same names everywhere) ----
    d = {}
    d["ids"] = nc.dram_tensor("ids", [1, S], F32, kind="ExternalInput")
    d["sel"] = nc.dram_tensor("sel", [128, 2, 128], F16, kind="ExternalInput")
    d["embT"] = nc.dram_tensor("embT", [V, E], F16, kind="ExternalInput")
    d["wih0T"] = nc.dram_tensor("wih0T", [128, 2, MC, 128], F16,
                                kind="ExternalInput")
    d["wihT"] = nc.dram_tensor("wihT", [2, 128, 2, KC_H2, MC, 128], F16,
                               kind="ExternalInput")
    d["whhT"] = nc.dram_tensor("whhT", [NL, 128, 2, KC_H, MC, 128], F16,
                               kind="ExternalInput")
    d["biasT"] = nc.dram_tensor("biasT", [2, NL, 2, MC // 2, 128], F16,
                                kind="ExternalInput")
    d["ones2"] = nc.dram_tensor("ones2", [2, 2, 256], F16,
                                kind="ExternalInput")
    d["attnT"] = nc.dram_tensor("attnT", [128, 2, KC_H2, KC_H2, 128], F16,
                                kind="ExternalInput")
    d["vT"] = nc.dram_tensor("vT", [128, KC_H2], F16, kind="ExternalInput")
    d["vsel"] = nc.dram_tensor("vsel", [128, KC_H2, 32, 32], F16,
                               kind="ExternalInput")
    d["clsWT"] = nc.dram_tensor("clsWT", [128, KC_H2, C], F16,
                                kind="ExternalInput")
    d["clsb"] = nc.dram_tensor("clsb", [C, 1], F32, kind="ExternalInput")
    d["id16"] = nc.dram_tensor("id16", [128, 128], F16, kind="ExternalInput")
    d["id32"] = nc.dram_tensor("id32", [128, 128], F32, kind="ExternalInput")
    d["out"] = nc.dram_tensor("logitsT", [C, 128], F32, kind="ExternalOutput")

    with tile.TileContext(nc) as tc, ExitStack() as ctx:
        _emit(ctx, tc, nc, nl, s_steps, d)
    nc.compile()
    return nc


def _emit(ctx, tc, nc, nl, SS, d):
    fp = ctx.enter_context(tc.tile_pool(name="persist", bufs=1))

    # ---- load constants to SBUF, ordered so layer 0 can start ASAP ----
    def _alloc(name, shape, dtype):
        return fp.tile(shape, dtype, name=f"sb_{name}", tag=f"sb_{name}")

    def _dma(t, name, sl=None):
        if sl is None:
            nc.gpsimd.dma_start(out=t[:], in_=d[name][:])
        else:
            nc.gpsimd.dma_start(out=t[:, sl], in_=d[name][:, sl])

    emb_sb = _alloc("embT", [V, E], F16)
    wih0_sb = _alloc("wih0T", [128, 2, MC, 128], F16)
    whh_sb = _alloc("whhT", [128, NL, 2, KC_H, MC, 128], F16)
    bias_sb = _alloc("biasT", [2, NL, 2, MC // 2, 128], F16)
    ones2_sb = _alloc("ones2", [2, 2, 256], F16)
    wih_sb = _alloc("wihT", [128, 2, 2, KC_H2, MC, 128], F16)
    attn_sb = _alloc("attnT", [128, 2, KC_H2, KC_H2, 128], F16)
    v_sb = _alloc("vT", [128, KC_H2], F16)
    vsel_sb = _alloc("vsel", [128, KC_H2, 32, 32], F16)
    clsw_sb = _alloc("clsWT", [128, KC_H2, C], F16)
    clsb_sb = _alloc("clsb", [C, 1], F32)
    id16_sb = _alloc("id16", [128, 128], F16)
    id32_sb = _alloc("id32", [128, 128], F32)
    sel_sb = _alloc("sel", [128, 2, 128], F16)

    # ids broadcast (64 stride-0 descriptors) on the SP queue so it
    # doesn't delay the weight stream on the gpsimd queue
    ids_ap = d["ids"].ap()
    ids_b = fp.tile([V, S], F32)
    nc.sync.dma_start(
        out=ids_b[:],
        in_=bass.AP(tensor=ids_ap.tensor, offset=ids_ap.offset,
                    ap=[[0, V], [1, S]]),
    )
    # early: everything layer 0 needs (whhT/wihT are layer-major in DRAM
    # so per-layer slices are contiguous, not 128 strided descriptors)
    _dma(emb_sb, "embT")
    _dma(wih0_sb, "wih0T")
    _dma(bias_sb, "biasT")
    _dma(ones2_sb, "ones2")
    _dma(id16_sb, "id16")
    nc.gpsimd.dma_start(out=whh_sb[:, 0], in_=d["whhT"][0])
    nc.gpsimd.dma_start(out=whh_sb[:, 1], in_=d["whhT"][1])
    nc.gpsimd.dma_start(out=wih_sb[:, 0], in_=d["wihT"][0])
    nc.gpsimd.dma_start(out=whh_sb[:, 2], in_=d["whhT"][2])
    nc.gpsimd.dma_start(out=wih_sb[:, 1], in_=d["wihT"][1])
    # late: attention/classifier-phase tensors on the idle SP queue
    nc.sync.dma_start(out=attn_sb[:], in_=d["attnT"][:])
    nc.sync.dma_start(out=v_sb[:], in_=d["vT"][:])
    nc.sync.dma_start(out=vsel_sb[:], in_=d["vsel"][:])
    nc.sync.dma_start(out=clsw_sb[:], in_=d["clsWT"][:])
    nc.sync.dma_start(out=clsb_sb[:], in_=d["clsb"][:])
    nc.sync.dma_start(out=id32_sb[:], in_=d["id32"][:])
    nc.sync.dma_start(out=sel_sb[:], in_=d["sel"][:])

    zeros16 = fp.tile([128, KC_H], F16)
    nc.vector.memset(zeros16[:], 0.0)
    neg2 = fp.tile([128, 1], F32)
    nc.vector.memset(neg2[:], -2.0)

    # ---- embedding: one-hot matmul -> xT [E=128, S] f16 ----
    iota_i = fp.tile([V, 1], mybir.dt.int32)
    nc.gpsimd.iota(iota_i[:], pattern=[[0, 1]], base=0, channel_multiplier=1)
    iota_f = fp.tile([V, 1], F32)
    nc.vector.tensor_copy(iota_f[:], iota_i[:])
    oh = fp.tile([V, S], F16)
    nc.vector.tensor_scalar(
        out=oh[:], in0=ids_b[:], scalar1=iota_f[:], scalar2=None,
        op0=OP.is_equal,
    )
    xT_sb = fp.tile([128, 1, S], F16)   # layer-0 input, 1 k-chunk
    with tc.tile_pool(name="embp", bufs=1, space="PSUM") as embp:
        x_ps = embp.tile([128, S], F32)
        nc.tensor.matmul(x_ps[:], emb_sb[:], oh[:], start=True, stop=True)
        nc.vector.tensor_copy(xT_sb[:, 0, :], x_ps[:])

    # ---- LSTM layers ----
    h16_pool = ctx.enter_context(tc.tile_pool(name="h16", bufs=2))
    prev = xT_sb          # [128, kc_in, S] f16
    kc_in = 1
    for layer in range(nl):
        wl = wih0_sb if layer == 0 else wih_sb
        hT16 = h16_pool.tile([128, 4, S], F16, tag="hT16")
        with (
            tc.tile_pool(name=f"gps{layer}", bufs=1, space="PSUM") as gps,
            tc.tile_pool(name=f"rsb{layer}", bufs=4) as rsb,
        ):
            # gate pre-activations, whole layer, in PSUM (all 8 banks)
            g_ps = gps.tile([128, 2, MC, S], F32)
            # bias via selector matmul: one per (dir, mc-pair) -> 1 bank
            for dd in (0, 1):
                for p in range(MC // 2):
                    nc.tensor.matmul(
                        g_ps[:, dd, 2 * p:2 * p + 2, :],
                        bias_sb[:, layer, dd, p, :], ones2_sb[:],
                        start=True, stop=True, skip_group_check=True,
                    )
            # input GEMM accumulates on top
            for dd in (0, 1):
                for mc in range(MC):
                    for kc in range(kc_in):
                        if layer == 0:
                            lhsT = wl[:, dd, mc, :]
                        else:
                            lhsT = wl[:, layer - 1, dd, kc, mc, :]
                        nc.tensor.matmul(
                            g_ps[:, dd, mc, :], lhsT, prev[:, kc, :],
                            start=False, stop=(kc == kc_in - 1),
                            skip_group_check=True,
                        )
            # recurrence: fwd (d=0) + bwd (d=1) interleaved chains.
            # Cell state is carried as D = c/2 + 1/2 in cols 8:10 of the
            # per-step gate tile st = [sig(i) sig(f) sig(o) sig(2g) | D],
            # so ONE scalar_tensor_tensor of form (x-1/2)*y yields both
            # i*tanh(g)/2 = (sig(2g)-1/2)*sig(i) and f*c/2 = (D-1/2)*sig(f),
            # a second stt folds the +1/2 into the sum, and tanh(c) =
            # 2*sig(2c)-1 comes from sig(4D-2) via ACT scale+bias.
            # 3 DVE ops + 2 ACT ops per dir-step, 5 serial stages.
            st = [[None, None], [None, None]]
            for dd in (0, 1):
                for par in (0, 1):
                    st[dd][par] = rsb.tile([128, 10], F32,
                                           tag=f"st{dd}{par}",
                                           name=f"st{dd}{par}_{layer}")
                nc.vector.memset(st[dd][0][:, 8:10], 0.5)
            for t in range(SS):
                for dd in (0, 1):
                    tg = t if dd == 0 else SS - 1 - t
                    tprev = tg + 1 if dd == 1 else tg - 1
                    for mc in range(MC):
                        for kc in range(KC_H):
                            if t == 0:
                                rhs = zeros16[:, kc:kc + 1]
                            else:
                                rhs = hT16[:, 2 * dd + kc, tprev:tprev + 1]
                            nc.tensor.matmul(
                                g_ps[:, dd, mc, tg:tg + 1],
                                whh_sb[:, layer, dd, kc, mc, :], rhs,
                                start=False,
                                stop=(mc == MC - 1 and kc == KC_H - 1),
                                skip_group_check=True,
                            )
                    # gates: i=0:2 f=2:4 o=4:6 g=6:8 (chunk cols).
                    # g-rows pre-scaled x2 host-side; all gate weights
                    # carry x16, undone via the activation scale.
                    cur = st[dd][t % 2]
                    nxt = st[dd][(t + 1) % 2]
                    nc.scalar.activation(cur[:, 0:8], g_ps[:, dd, :, tg],
                                         AF.Sigmoid, scale=1.0 / GSC)
                    prods = rsb.tile([128, 4], F32, tag=f"p{dd}")
                    nc.vector.scalar_tensor_tensor(
                        out=prods[:], in0=cur[:, 6:10], scalar=0.5,
                        in1=cur[:, 0:4], op0=OP.subtract, op1=OP.mult)
                    nc.vector.scalar_tensor_tensor(
                        out=nxt[:, 8:10], in0=prods[:, 0:2], scalar=0.5,
                        in1=prods[:, 2:4], op0=OP.add, op1=OP.add)
                    tc_t = rsb.tile([128, 2], F32, tag=f"tc{dd}")
                    nc.scalar.activation(tc_t[:], nxt[:, 8:10], AF.Sigmoid,
                                         scale=4.0, bias=neg2[:])
                    nc.vector.scalar_tensor_tensor(
                        out=hT16[:, 2 * dd:2 * dd + 2, tg], in0=tc_t[:],
                        scalar=0.5, in1=cur[:, 4:6],
                        op0=OP.subtract, op1=OP.mult)
        prev = hT16
        kc_in = KC_H2

    # ---- attention + classifier ----
    hT = prev  # [128, 4, S] f16 final hidden (transposed layout)
    ap1 = ctx.enter_context(tc.tile_pool(name="attn1", bufs=1))

    # h layout [s, h]: PE-transpose hT blocks -> h_sb[:, sc, hc, :]
    h_sb = ap1.tile([128, 2, KC_H2, 128], F16)
    with tc.tile_pool(name="trps", bufs=4, space="PSUM") as trps:
        for hc in range(KC_H2):
            for sc in range(2):
                tp = trps.tile([128, 128], F16, tag="tp")
                nc.tensor.transpose(tp[:], hT[:, hc, sc * 128:(sc + 1) * 128],
                                    id16_sb[:])
                nc.vector.tensor_copy(h_sb[:, sc, hc, :], tp[:])

    # hT_sel [h, tl] = h-cols for my t-half, via Sel matmul; then q, k
    hsel_sb = ap1.tile([128, KC_H2, 128], F16)
    qT_sb = ap1.tile([128, KC_H2, 128], F32)
    with ExitStack() as actx:
        kTp = actx.enter_context(tc.tile_pool(name="kTps", bufs=1, space="PSUM"))
        kT_ps = kTp.tile([128, KC_H2, S], F32)
        with tc.tile_pool(name="qkps", bufs=2, space="PSUM") as qkps:
            for hc in range(KC_H2):
                ps = qkps.tile([128, 128], F32, tag="sel")
                for sc in range(2):
                    nc.tensor.matmul(ps[:], h_sb[:, sc, hc, :],
                                     sel_sb[:, sc, :],
                                     start=(sc == 0), stop=(sc == 1))
                nc.vector.tensor_copy(hsel_sb[:, hc, :], ps[:])
            for mc in range(KC_H2):
                psq = qkps.tile([128, 128], F32, tag="q")
                for kc in range(KC_H2):
                    nc.tensor.matmul(psq[:], attn_sb[:, 0, kc, mc, :],
                                     hsel_sb[:, kc, :],
                                     start=(kc == 0), stop=(kc == KC_H2 - 1))
                nc.vector.tensor_copy(qT_sb[:, mc, :], psq[:])
            for mc in range(KC_H2):
                for kc in range(KC_H2):
                    nc.tensor.matmul(kT_ps[:, mc, :], attn_sb[:, 1, kc, mc, :],
                                     hT[:, kc, :],
                                     start=(kc == 0), stop=(kc == KC_H2 - 1))

        # scores[t, s] = sum_hc vT . tanh(kT + q[t]); 32 t-rows per psum
        # tile via v-selector lhsT (v in col t%32, zero rows accumulate 0)
        scp = actx.enter_context(tc.tile_pool(name="scps", bufs=2, space="PSUM"))
        scores_sb = ap1.tile([128, S], F32)
        with tc.tile_pool(name="tanhp", bufs=4) as tanhp:
            for tg_i in range(4):
                sc_ps = scp.tile([32, S], F32, tag="sc")
                for tj in range(32):
                    t = tg_i * 32 + tj
                    for hc in range(KC_H2):
                        th_t = tanhp.tile([128, S], F16, tag="th")
                        nc.scalar.activation(th_t[:], kT_ps[:, hc, :], AF.Tanh,
                                             bias=qT_sb[:, hc, t:t + 1])
                        nc.tensor.matmul(
                            sc_ps[:], vsel_sb[:, hc, tj, :], th_t[:],
                            start=(tj == 0 and hc == 0),
                            stop=(tj == 31 and hc == KC_H2 - 1),
                            skip_group_check=True,
                        )
                nc.vector.tensor_copy(
                    scores_sb[tg_i * 32:(tg_i + 1) * 32, :], sc_ps[:])

        # softmax rows -> wn_sb [128, S] f32
        ap2 = ctx.enter_context(tc.tile_pool(name="attn2", bufs=1))
        wn_sb = ap2.tile([128, S], F32)
        nmax = ap2.tile([128, 1], F32)
        nc.vector.tensor_reduce(out=nmax[:], in_=scores_sb[:], op=OP.max,
                                axis=mybir.AxisListType.X, negate=True)
        rsum = ap2.tile([128, 1], F32)
        wexp = ap2.tile([128, S], F32)
        nc.scalar.activation(wexp[:], scores_sb[:], AF.Exp,
                             bias=nmax[:], accum_out=rsum[:])
        rinv = ap2.tile([128, 1], F32)
        nc.vector.reciprocal(rinv[:], rsum[:])
        nc.vector.tensor_scalar_mul(wn_sb[:], wexp[:], rinv[:])

    # wT via PE transpose -> f16; ctxT; classifier
    ap2b = ctx.enter_context(tc.tile_pool(name="attn3", bufs=1))
    wT_sb = ap2b.tile([128, 2, 128], F16)
    ctxT_sb = ap2b.tile([128, KC_H2, 128], F16)
    with tc.tile_pool(name="ctps", bufs=2, space="PSUM") as ctps:
        for sc in range(2):
            tp32 = ctps.tile([128, 128], F32, tag="wt")
            nc.tensor.transpose(tp32[:], wn_sb[:, sc * 128:(sc + 1) * 128],
                                id32_sb[:])
            nc.vector.tensor_copy(wT_sb[:, sc, :], tp32[:])
        for hc in range(KC_H2):
            ps = ctps.tile([128, 128], F32, tag="ctx")
            for sc in range(2):
                nc.tensor.matmul(ps[:], h_sb[:, sc, hc, :], wT_sb[:, sc, :],
                                 start=(sc == 0), stop=(sc == 1))
            nc.vector.tensor_copy(ctxT_sb[:, hc, :], ps[:])
        lps = ctps.tile([C, 128], F32, tag="log")
        for kc in range(KC_H2):
            nc.tensor.matmul(lps[:], clsw_sb[:, kc, :], ctxT_sb[:, kc, :],
                             start=(kc == 0), stop=(kc == KC_H2 - 1))
        lsb = ap2b.tile([C, 128], F32)
        nc.vector.tensor_scalar_add(out=lsb[:], in0=lps[:], scalar1=clsb_sb[:])
        nc.sync.dma_start(out=d["out"][:], in_=lsb[:])


# ---------------- host side ----------------

def _prep_inputs(inputs):
    """Per-core input maps from the full problem inputs."""
    ids = np.asarray(inputs["input_ids"])
    emb = np.asarray(inputs["emb"], np.float32)
    w_ih0 = np.asarray(inputs["w_ih0"], np.float32)[:, _PERM, :].copy()
    w_hh0 = np.asarray(inputs["w_hh0"], np.float32)[:, _PERM, :].copy()
    b0 = np.asarray(inputs["b0"], np.float32)[:, _PERM].copy()
    w_ih = np.asarray(inputs["w_ih"], np.float32)[:, :, _PERM, :].copy()
    w_hh = np.asarray(inputs["w_hh"], np.float32)[:, :, _PERM, :].copy()
    b = np.asarray(inputs["b"], np.float32)[:, :, _PERM].copy()
    # tanh-as-sigmoid identity: scale g-gate rows x2
    w_ih0[:, 768:] *= 2.0
    w_hh0[:, 768:] *= 2.0
    b0[:, 768:] *= 2.0
    w_ih[:, :, 768:] *= 2.0
    w_hh[:, :, 768:] *= 2.0
    b[:, :, 768:] *= 2.0
    attn_W = np.asarray(inputs["attn_W"], np.float32)
    attn_U = np.asarray(inputs["attn_U"], np.float32)
    attn_v = np.asarray(inputs["attn_v"], np.float32)
    cls_W = np.asarray(inputs["cls_W"], np.float32)
    cls_b = np.asarray(inputs["cls_b"], np.float32)

    wih0T = np.empty((128, 2, MC, 128), np.float32)
    for dd in range(2):
        wih0T[:, dd] = w_ih0[dd].T.reshape(E, MC, 128)
    # layer-major in DRAM so per-layer DMA slices are contiguous
    wihT = np.empty((2, 128, 2, KC_H2, MC, 128), np.float32)
    for li in range(2):
        for dd in range(2):
            wihT[li, :, dd] = (w_ih[li, dd].T.reshape(KC_H2, 128, MC, 128)
                               .transpose(1, 0, 2, 3))
    whhT = np.empty((NL, 128, 2, KC_H, MC, 128), np.float32)
    for layer in range(NL):
        for dd in range(2):
            wt = (w_hh0[dd] if layer == 0 else w_hh[layer - 1, dd]).T
            whhT[layer, :, dd] = (wt.reshape(KC_H, 128, MC, 128)
                                  .transpose(1, 0, 2, 3))
    # bias as selector-matmul lhsT: [2, layer, dir, mc-pair, 128]
    biasT = np.empty((2, NL, 2, MC // 2, 128), np.float32)
    for layer in range(NL):
        for dd in range(2):
            bb = b0[dd] if layer == 0 else b[layer - 1, dd]
            biasT[:, layer, dd] = bb.reshape(MC // 2, 2, 128).transpose(1, 0, 2)
    ones2 = np.zeros((2, 2, 256), np.float16)
    ones2[0, 0, :] = 1.0
    ones2[1, 1, :] = 1.0
    attnT = np.empty((128, 2, KC_H2, KC_H2, 128), np.float32)
    for i, m in enumerate((attn_W, attn_U)):
        attnT[:, i] = (m.T.reshape(KC_H2, 128, KC_H2, 128)
                       .transpose(1, 0, 2, 3))
    vT = attn_v.reshape(KC_H2, 128).T.astype(np.float16)
    vsel = np.zeros((128, KC_H2, 32, 32), np.float16)
    for tj in range(32):
        vsel[:, :, tj, tj] = vT

    clsWT = cls_W.T.reshape(KC_H2, 128, C).transpose(1, 0, 2).astype(np.float16)
    clsb = cls_b.reshape(C, 1).astype(np.float32)
    id16 = np.eye(128, dtype=np.float16)
    id32 = np.eye(128, dtype=np.float32)

    # h is stored as h/2 on device; double every matrix whose input is h
    wihT *= 2.0
    whhT *= 2.0
    attnT *= 2.0
    clsWT *= 2.0
    # gate pre-activations carry x16 (kept from the fp8 experiments, the
    # sigmoid applies 1/16; harmless in f16 since |g*16| < 512)
    wih0T *= GSC
    wihT *= GSC
    whhT *= GSC
    biasT *= GSC
    common = dict(
        embT=emb.astype(np.float16),
        wih0T=wih0T.astype(np.float16),
        wihT=wihT.astype(np.float16),
        whhT=whhT.astype(np.float16),
        biasT=biasT.astype(np.float16),
        ones2=ones2,
        attnT=attnT.astype(np.float16),
        vT=vT, vsel=vsel, clsWT=clsWT, clsb=clsb,
        id16=id16, id32=id32,
    )
    in_maps = []
    for c in range(N_CORES):
        bb, th = c // 2, c % 2
        sel = np.zeros((S, 128), np.float16)
        sel[np.arange(128) + th * 128, np.arange(128)] = 1.0
        m = dict(common)
        m["ids"] = ids[bb].astype(np.float32).reshape(1, S)
        m["sel"] = sel.reshape(2, 128, 128).transpose(1, 0, 2).copy()
        in_maps.append(m)
    return in_maps


_NC_CACHE = {}


def _get_nc():
    if "nc" not in _NC_CACHE:
        _NC_CACHE["nc"] = _build_nc()
    return _NC_CACHE["nc"]


def kernel(**inputs) -> np.ndarray:
    from concourse.bass_utils import run_bass_kernel_spmd

    nc = _get_nc()
    in_maps = _prep_inputs(inputs)
    res = run_bass_kernel_spmd(nc, in_maps, list(range(N_CORES)))
    out = np.empty((B, S, C), np.float32)
    for c in range(N_CORES):
        bb, th = c // 2, c % 2
        out[bb, th * 128:(th + 1) * 128, :] = res.results[c]["logitsT"].T
    return out
